# revision 1
# baseline (speedup 1.0000x reference)
"""InfoNCE loss kernel for 8 Trainium2 NeuronCores (fp8 DoubleRow version).

Math (reference): z = concat(z1, z2) [2N, D] row-normalized; sim = z@z.T/TEMP;
self-diagonal masked; loss = mean(-pos + logsumexp(sim, axis=1)) where
pos[i] = sim[i, partner(i)].

Sharding: data-parallel over the 2N row dimension - core c owns rows
[c*1024, (c+1)*1024). Each core computes its [1024, 8192] block of sim
against the full z with fp8e4m3 DoubleRow matmuls (256-deep contraction per
instruction -> 512 matmuls/core), applies exp(x/TEMP - 1/TEMP) on the scalar
engine over two PSUM banks at a time, row-reduces on the vector engine,
extracts the positive diagonal from raw PSUM, and returns per-row
(ln(S_r) - pos_r). Host adds the constant 1/TEMP shift and takes the mean.

Tricks:
- columns of z are permuted per-core so the self block is always block 0 and
  the positive-partner block is always block 1, making the SPMD graph
  identical across cores (diag offsets are compile-time constants).
- z is pre-scaled by 8 on the host before the fp8 cast (uses the e4m3
  dynamic range); the 1/64 is folded into the exp scale.
"""

from contextlib import ExitStack

import ml_dtypes
import numpy as np

import concourse.bass as bass
import concourse.tile as tile
from concourse import bacc, mybir
from concourse.bass_utils import run_bass_kernel_spmd
from concourse.masks import make_identity

N_CORES = 8
N, D = 4096, 1024
ROWS = 2 * N               # 8192 total rows of z
RPC = ROWS // N_CORES      # 1024 rows per core
TEMP = 0.07
INV_T = 1.0 / TEMP
FP8_SCALE = 8.0            # host pre-scale before e4m3 cast
MM_SCALE = INV_T / (FP8_SCALE * FP8_SCALE)
NTILE = 512                # columns per z SBUF tile / PSUM bank (fp32)
NT = ROWS // NTILE         # 16 column tiles
MT = RPC // 128            # 8 row tiles per core
KT = D // 128              # 8 contraction slices (4 DoubleRow pairs)
NPAIR = NT // 2            # 8 column-tile pairs per row tile

_CACHE = {}


def _build_graph():
    nc = bacc.Bacc("TRN2", target_bir_lowering=False, debug=False, num_devices=N_CORES)
    z = nc.declare_dram_parameter("z", [NT, 128, KT, NTILE], mybir.dt.float8e4, isOutput=False)
    out = nc.declare_dram_parameter("out", [128, MT], mybir.dt.float32, isOutput=True)

    fp32 = mybir.dt.float32
    bf16 = mybir.dt.bfloat16
    fp8 = mybir.dt.float8e4
    AF = mybir.ActivationFunctionType
    AX = mybir.AxisListType.X
    DR = mybir.MatmulPerfMode.DoubleRow

    with tile.TileContext(nc) as tc, ExitStack() as ctx:
        zpool = ctx.enter_context(tc.tile_pool(name="z", bufs=1))
        consts = ctx.enter_context(tc.tile_pool(name="consts", bufs=1))
        pspool = ctx.enter_context(tc.tile_pool(name="ps", bufs=4, space="PSUM"))
        expool = ctx.enter_context(tc.tile_pool(name="ex", bufs=6))
        pmpool = ctx.enter_context(tc.tile_pool(name="pm", bufs=4))
        accpool = ctx.enter_context(tc.tile_pool(name="acc", bufs=3))
        outpool = ctx.enter_context(tc.tile_pool(name="outp", bufs=1))

        # stage z into SBUF first: one [128, KT, 512] fp8 tile per column
        # block, one contiguous 512KB DMA each - issued before anything else
        # so the transfers lead the compute
        zc = []
        for c in range(NT):
            t = zpool.tile([128, KT, NTILE], fp8, tag=f"zc{c}", name=f"zc{c}")
            # odd blocks 1..9 ride the scalar engine's DGE queue: two
            # transfer streams while the scalar engine is still idle, so the
            # first matmul pairs aren't DMA-paced
            eng = nc.scalar if (c % 2 == 1 and c < 10) else nc.sync
            eng.dma_start(out=t[:], in_=z[c])
            zc.append(t)

        # constants: identity (positive extraction), 1-identity (self mask,
        # bf16 to match the exp tiles), bias column of -1/TEMP
        eye = consts.tile([128, 128], fp32, tag="eye")
        make_identity(nc, eye[:])
        aeye = consts.tile([128, 128], bf16, tag="aeye")
        nc.gpsimd.memset(aeye[:], 1.0)
        nc.gpsimd.affine_select(
            out=aeye[:],
            in_=aeye[:],
            compare_op=mybir.AluOpType.not_equal,
            fill=0.0,
            base=0,
            pattern=[[-1, 128]],
            channel_multiplier=1,
        )
        nbias = consts.tile([128, 1], fp32, tag="nbias")
        nc.vector.memset(nbias[:], -INV_T)

        Scol = outpool.tile([128, MT], fp32, tag="Scol")      # per-row exp sums
        dotcols = outpool.tile([128, MT], fp32, tag="dotcols")  # raw positive dots
        outsb = outpool.tile([128, MT], fp32, tag="outsb")

        for mt in range(MT):
            cself = mt // 4              # column tile holding this m-tile's diag
            off = (mt % 4) * 128         # diag offset within that 512-wide tile
            # pair 0 (cols 0..1023) holds the self diag; pair 1 (cols
            # 1024..2047) holds the positive-partner diag
            dofh = cself                 # which half of the pair tile
            acc = accpool.tile([128, NPAIR], fp32, tag="acc")

            for p in range(NPAIR):
                ps = pspool.tile([128, 2, NTILE], fp32, tag="ps", name="ps")
                for k in range(KT // 2):
                    lt = zc[cself][:, 2 * k : 2 * k + 2, off : off + 128]
                    for h in range(2):
                        nc.tensor.matmul(
                            ps[:, h, :],
                            lhsT=lt,
                            rhs=zc[2 * p + h][:, 2 * k : 2 * k + 2, :],
                            start=(k == 0),
                            stop=(k == KT // 2 - 1),
                            perf_mode=DR,
                        )
                if p == 1:
                    # positive-partner diag: extract raw dot from PSUM
                    pm = pmpool.tile([128, 128], fp32, tag="pm")
                    nc.vector.tensor_mul(pm[:], ps[:, dofh, off : off + 128], eye[:])
                    nc.vector.reduce_sum(dotcols[:, mt : mt + 1], pm[:], axis=AX)
                ex = expool.tile([128, 2, NTILE], bf16, tag="ex")
                if p == 0:
                    # self pair: exp, zero the self-diagonal, then row-sum on DVE
                    nc.scalar.activation(
                        out=ex[:], in_=ps[:], func=AF.Exp, bias=nbias[:], scale=MM_SCALE
                    )
                    nc.vector.tensor_mul(
                        ex[:, dofh, off : off + 128],
                        ex[:, dofh, off : off + 128],
                        aeye[:],
                    )
                    nc.vector.reduce_sum(
                        acc[:, 0:1], ex[:], axis=mybir.AxisListType.XY
                    )
                else:
                    # fused exp + row-sum on the scalar engine
                    nc.scalar.activation(
                        out=ex[:], in_=ps[:], func=AF.Exp, bias=nbias[:],
                        scale=MM_SCALE, accum_out=acc[:, p : p + 1],
                    )

            nc.vector.reduce_sum(Scol[:, mt : mt + 1], acc[:], axis=AX)

        # tail: ln(S) and combine (kept out of the loop so the scalar engine
        # doesn't thrash activation tables between Exp and Ln)
        lnS = outpool.tile([128, MT], fp32, tag="lnS")
        nc.scalar.activation(out=lnS[:], in_=Scol[:], func=AF.Ln, bias=0.0, scale=1.0)
        dsc = outpool.tile([128, MT], fp32, tag="dsc")
        nc.scalar.activation(
            out=dsc[:], in_=dotcols[:], func=AF.Identity, bias=0.0, scale=-MM_SCALE
        )
        nc.vector.tensor_add(outsb[:], lnS[:], dsc[:])
        nc.sync.dma_start(out=out[:], in_=outsb[:])

    nc.compile()
    return nc


def _make_in_maps(z1: np.ndarray, z2: np.ndarray):
    z = np.concatenate([z1, z2], axis=0)  # [8192, 1024] f32
    # per-core column permutation: [self block, partner block, rest]
    in_maps = []
    zs = (z * FP8_SCALE).astype(np.float32)
    for c in range(N_CORES):
        p = (c + 4) % N_CORES
        order = [c, p] + [b for b in range(N_CORES) if b != c and b != p]
        idx = np.concatenate([np.arange(b * RPC, (b + 1) * RPC) for b in order])
        zt = zs[idx].T  # [D, ROWS] permuted
        # [NT, 128, KT, NTILE]: per column-block, contiguous [p, k, n] tiles
        zcb = np.ascontiguousarray(
            zt.reshape(KT, 128, NT, NTILE).transpose(2, 1, 0, 3)
        ).astype(ml_dtypes.float8_e4m3)
        in_maps.append({"z": zcb})
    return in_maps


def kernel(z1: np.ndarray, z2: np.ndarray) -> np.ndarray:
    assert z1.shape == (N, D) and z2.shape == (N, D)
    in_maps = _make_in_maps(z1, z2)

    if "nc" not in _CACHE:
        _CACHE["nc"] = _build_graph()
    res = run_bass_kernel_spmd(_CACHE["nc"], in_maps, core_ids=list(range(N_CORES)))

    total = 0.0
    for r in res.results:
        total += float(np.asarray(r["out"], dtype=np.float64).sum())
    return np.asarray(INV_T + total / ROWS, dtype=np.float32)



# revision 10
# speedup vs baseline: 1.5621x; 1.5621x over previous
"""InfoNCE loss kernel for 8 Trainium2 NeuronCores (symmetric-triangle version).

Math (reference): z = concat(z1, z2) [2N, D] row-normalized; sim = z@z.T/TEMP;
self-diagonal masked; loss = mean(-pos + logsumexp(sim, axis=1)).

sim is SYMMETRIC, so only a triangle of the 16x16 grid of 512-wide band
blocks is computed: 136 blocks instead of 256. Core c (with per-core band
rotation slot s -> band (c+s)%16) computes the canonical pattern
  lhs slot 0:  rhs slots 0..8   (slot 0 = self-diagonal block)
  lhs slot 8:  rhs slots 8..15  (slot 8 = self-diagonal block)
which covers every unordered band pair exactly once across the 8 cores.
Each off-diagonal block contributes exp row-sums (fused scalar-engine
accum_out) to its lhs band AND exp column-sums (fp8e5m2 DoubleRow
ones-matmul over the partition axis) to its rhs band. Diagonal blocks are
masked with (1-I) after exp and row-reduced on the vector engine. The host
sums the per-core partial sums, takes ln, and subtracts the exactly-computed
positive dots. This halves the tensor-engine work vs the full-sim version
(302 DoubleRow matmuls/core vs 512).

Tricks kept from the full-sim version: z pre-scaled by 8 before the e4m3
cast (1/64 folded into the exp scale); fp8 DoubleRow 256-deep contraction.
Column-sum matmul emission is deferred past the next pair's first chains so
the PE never stalls waiting on the scalar engine's exp.
"""

from contextlib import ExitStack

import ml_dtypes
import numpy as np

import concourse.bass as bass
import concourse.tile as tile
from concourse import bacc, mybir
from concourse.bass_utils import run_bass_kernel_spmd

N_CORES = 8
N, D = 4096, 1024
ROWS = 2 * N               # 8192 rows of z
NB = 16                    # 512-row bands
BAND = ROWS // NB          # 512
KT = D // 128              # 8 contraction slices (4 DoubleRow pairs)
TEMP = 0.07
INV_T = 1.0 / TEMP
FP8_SCALE = 8.0            # host pre-scale before e4m3 cast
MM_SCALE = INV_T / (FP8_SCALE * FP8_SCALE)

_CACHE = {}


def _build_graph():
    nc = bacc.Bacc("TRN2", target_bir_lowering=False, debug=False, num_devices=N_CORES)
    z = nc.declare_dram_parameter("z", [NB, 128, KT, BAND], mybir.dt.float8e4, isOutput=False)
    rowacc_d = nc.declare_dram_parameter("rowacc", [128, 2, 4, 4], mybir.dt.float32, isOutput=True)
    diagacc_d = nc.declare_dram_parameter("diagacc", [128, 2, 4], mybir.dt.float32, isOutput=True)
    colsum_d = nc.declare_dram_parameter("colsum", [1, 15, BAND], mybir.dt.float32, isOutput=True)

    fp32 = mybir.dt.float32
    bf16 = mybir.dt.bfloat16
    fp8e4 = mybir.dt.float8e4
    fp8e5 = mybir.dt.float8e5
    AF = mybir.ActivationFunctionType
    AX = mybir.AxisListType.X
    DR = mybir.MatmulPerfMode.DoubleRow

    with tile.TileContext(nc) as tc, ExitStack() as ctx:
        zpool = ctx.enter_context(tc.tile_pool(name="z", bufs=1))
        consts = ctx.enter_context(tc.tile_pool(name="consts", bufs=1))
        pspool = ctx.enter_context(tc.tile_pool(name="ps", bufs=3, space="PSUM"))
        cspool = ctx.enter_context(tc.tile_pool(name="cs", bufs=2, space="PSUM"))
        expool = ctx.enter_context(tc.tile_pool(name="ex", bufs=2))
        exdpool = ctx.enter_context(tc.tile_pool(name="exd", bufs=2))
        outpool = ctx.enter_context(tc.tile_pool(name="outp", bufs=1))

        # stage z into SBUF: one [128, KT, 512] fp8 tile per band slot, all on
        # the sync HWDGE queue in slot order so slot 0 (the first block's only
        # dependency) lands first and compute starts ~3us in
        zc = []
        for s in range(NB):
            t = zpool.tile([128, KT, BAND], fp8e4, tag=f"zc{s}", name=f"zc{s}")
            nc.sync.dma_start(out=t[:], in_=z[s])
            zc.append(t)

        # constants: 1-identity (self mask, bf16) and a fp8 ones column for
        # the DoubleRow column-sum matmuls
        aeye = consts.tile([128, 128], bf16, tag="aeye")
        nc.gpsimd.memset(aeye[:], 1.0)
        nc.gpsimd.affine_select(
            out=aeye[:],
            in_=aeye[:],
            compare_op=mybir.AluOpType.not_equal,
            fill=0.0,
            base=0,
            pattern=[[-1, 128]],
            channel_multiplier=1,
        )
        ones8 = consts.tile([128, 2, 128], fp8e5, tag="ones8")
        nc.gpsimd.memset(ones8[:], 1.0)

        rowacc = outpool.tile([128, 2, 4, 4], fp32, tag="rowacc")
        diagacc = outpool.tile([128, 2, 4], fp32, tag="diagacc")
        cs_sb = outpool.tile([128, 15, BAND], fp32, tag="cs_sb")

        def mm_chain(ps_slice, L, s, mm):
            # [128 rows, 512 cols] block tile: 4 DoubleRow matmuls, K=1024
            for kp in range(4):
                nc.tensor.matmul(
                    ps_slice,
                    lhsT=zc[L][:, 2 * kp : 2 * kp + 2, 128 * mm : 128 * mm + 128],
                    rhs=zc[s][:, 2 * kp : 2 * kp + 2, :],
                    start=(kp == 0),
                    stop=(kp == 3),
                    perf_mode=DR,
                )

        # deferred column-sum emission: tensor-engine instructions execute in
        # program order, so the ones-matmuls (which wait on the scalar engine's
        # exp) are emitted after the NEXT pair's first chains to avoid PE stalls
        pending_cs = []
        cs_state = {"idx": 0, "cur": None}

        def flush_cs():
            for fn in pending_cs:
                fn()
            pending_cs.clear()

        def emit_cs(exq, h):
            # ones-matmul along the partition axis: every output row equals
            # the column sums of the block's 512 rows; row 0 is kept
            ci = cs_state["idx"]
            cs_state["idx"] += 1
            cur = cspool.tile([128, BAND], fp32, tag="cs", name="cs")
            nc.tensor.matmul(
                cur[:],
                lhsT=ones8[:],
                rhs=exq[:, 0:2, h, :],
                start=True,
                stop=False,
                perf_mode=DR,
            )
            nc.tensor.matmul(
                cur[:],
                lhsT=ones8[:],
                rhs=exq[:, 2:4, h, :],
                start=False,
                stop=True,
                perf_mode=DR,
            )
            nc.vector.tensor_copy(cs_sb[0:1, ci, :], cur[0:1, :])

        def do_diag(d, L):
            # self block (slot L, slot L): exp, mask self-diagonal, row-reduce
            for mp in range(2):
                ps = pspool.tile([128, 2, BAND], fp32, tag="ps", name="ps")
                for h in range(2):
                    mm_chain(ps[:, h, :], L, L, 2 * mp + h)
                if mp == 0:
                    flush_cs()
                exd = exdpool.tile([128, 2, BAND], bf16, tag="exd")
                nc.scalar.activation(out=exd[:], in_=ps[:], func=AF.Exp, bias=0.0, scale=MM_SCALE)
                for h in range(2):
                    mm = 2 * mp + h
                    nc.vector.tensor_mul(
                        exd[:, h, 128 * mm : 128 * mm + 128],
                        exd[:, h, 128 * mm : 128 * mm + 128],
                        aeye[:],
                    )
                    nc.vector.reduce_sum(diagacc[:, d, mm : mm + 1], exd[:, h, :], axis=AX)

        def do_pair(d, L, slot, blocks):
            exq = expool.tile([128, 4, 2, BAND], fp8e5, tag="exq")
            if len(blocks) == 2:
                for mm in range(4):
                    ps = pspool.tile([128, 2, BAND], fp32, tag="ps", name="ps")
                    for h, s_ in enumerate(blocks):
                        mm_chain(ps[:, h, :], L, s_, mm)
                    if mm == 1:
                        flush_cs()
                    # fused exp + row-sum over both blocks' 1024 cols
                    nc.scalar.activation(
                        out=exq[:, mm, :, :], in_=ps[:], func=AF.Exp, bias=0.0,
                        scale=MM_SCALE, accum_out=rowacc[:, d, mm, slot : slot + 1],
                    )
            else:
                for mp in range(2):
                    ps = pspool.tile([128, 2, BAND], fp32, tag="ps", name="ps")
                    for h in range(2):
                        mm_chain(ps[:, h, :], L, blocks[0], 2 * mp + h)
                    if mp == 0:
                        flush_cs()
                    for h in range(2):
                        mm = 2 * mp + h
                        nc.scalar.activation(
                            out=exq[:, mm, 0, :], in_=ps[:, h, :], func=AF.Exp, bias=0.0,
                            scale=MM_SCALE, accum_out=rowacc[:, d, mm, slot : slot + 1],
                        )
            for h in range(len(blocks)):
                pending_cs.append(lambda exq=exq, h=h: emit_cs(exq, h))

        do_diag(0, 0)
        do_pair(0, 0, 0, [1, 2])
        do_pair(0, 0, 1, [3, 4])
        do_pair(0, 0, 2, [5, 6])
        do_pair(0, 0, 3, [7, 8])
        do_pair(1, 8, 0, [9, 10])
        do_pair(1, 8, 1, [11, 12])
        do_pair(1, 8, 2, [13, 14])
        do_pair(1, 8, 3, [15])
        do_diag(1, 8)
        flush_cs()

        nc.sync.dma_start(out=rowacc_d[:], in_=rowacc[:])
        nc.sync.dma_start(out=diagacc_d[:], in_=diagacc[:])
        nc.sync.dma_start(out=colsum_d[:], in_=cs_sb[0:1, :, :])

    nc.compile()
    return nc


def _make_in_maps(z1: np.ndarray, z2: np.ndarray):
    z = np.concatenate([z1, z2], axis=0)          # [8192, 1024] f32
    zt = (z.T * FP8_SCALE).astype(np.float32)     # [1024, 8192]
    # [NB, 128, KT, BAND] band-major fp8 tiles: band, k-within-tile, k-tile, col
    zb = np.ascontiguousarray(
        zt.reshape(KT, 128, NB, BAND).transpose(2, 1, 0, 3)
    ).astype(ml_dtypes.float8_e4m3)
    return [
        {"z": np.ascontiguousarray(zb[[(c + s) % NB for s in range(NB)]])}
        for c in range(N_CORES)
    ]


def kernel(z1: np.ndarray, z2: np.ndarray) -> np.ndarray:
    assert z1.shape == (N, D) and z2.shape == (N, D)
    in_maps = _make_in_maps(z1, z2)

    if "nc" not in _CACHE:
        _CACHE["nc"] = _build_graph()
    res = run_bass_kernel_spmd(_CACHE["nc"], in_maps, core_ids=list(range(N_CORES)))

    S = np.zeros(ROWS, np.float64)
    for c in range(N_CORES):
        r = res.results[c]
        ra = np.asarray(r["rowacc"], dtype=np.float64)    # [128, 2, 4, 4]
        da = np.asarray(r["diagacc"], dtype=np.float64)   # [128, 2, 4]
        cs = np.asarray(r["colsum"], dtype=np.float64)[0]  # [15, 512]
        for d, L in ((0, 0), (1, 8)):
            b = (c + L) % NB
            vals = ra[:, d, :, :].sum(axis=2) + da[:, d, :]   # [128 p, 4 m]
            S[BAND * b : BAND * (b + 1)] += vals.T.reshape(BAND)
        for ci, s in enumerate(list(range(1, 9)) + list(range(9, 16))):
            b = (c + s) % NB
            S[BAND * b : BAND * (b + 1)] += cs[ci, :]

    pos = (z1.astype(np.float64) * z2.astype(np.float64)).sum(axis=1) / TEMP
    loss = np.log(S).mean() - pos.mean()
    return np.asarray(loss, dtype=np.float32)


# revision 13
# speedup vs baseline: 1.5655x; 1.0022x over previous
"""InfoNCE loss kernel for 8 Trainium2 NeuronCores (symmetric-triangle version).

Math (reference): z = concat(z1, z2) [2N, D] row-normalized; sim = z@z.T/TEMP;
self-diagonal masked; loss = mean(-pos + logsumexp(sim, axis=1)).

sim is SYMMETRIC, so only a triangle of the 16x16 grid of 512-wide band
blocks is computed: 136 blocks instead of 256. Core c (with per-core band
rotation slot s -> band (c+s)%16) computes the canonical pattern
  lhs slot 0:  rhs slots 0..8   (slot 0 = self-diagonal block)
  lhs slot 8:  rhs slots 8..15  (slot 8 = self-diagonal block)
which covers every unordered band pair exactly once across the 8 cores.
Each off-diagonal block contributes exp row-sums (fused scalar-engine
accum_out) to its lhs band AND exp column-sums (fp8e5m2 DoubleRow
ones-matmul over the partition axis) to its rhs band. Diagonal blocks are
masked with (1-I) after exp and row-reduced on the vector engine. The host
sums the per-core partial sums, takes ln, and subtracts the exactly-computed
positive dots. This halves the tensor-engine work vs the full-sim version
(302 DoubleRow matmuls/core vs 512).

Tricks kept from the full-sim version: z pre-scaled by 8 before the e4m3
cast (1/64 folded into the exp scale); fp8 DoubleRow 256-deep contraction.
Column-sum matmul emission is deferred past the next pair's first chains so
the PE never stalls waiting on the scalar engine's exp.
"""

from contextlib import ExitStack

import ml_dtypes
import numpy as np

import concourse.bass as bass
import concourse.tile as tile
from concourse import bacc, mybir
from concourse.bass_utils import run_bass_kernel_spmd

N_CORES = 8
N, D = 4096, 1024
ROWS = 2 * N               # 8192 rows of z
NB = 16                    # 512-row bands
BAND = ROWS // NB          # 512
KT = D // 128              # 8 contraction slices (4 DoubleRow pairs)
TEMP = 0.07
INV_T = 1.0 / TEMP
FP8_SCALE = 8.0            # host pre-scale before e4m3 cast
MM_SCALE = INV_T / (FP8_SCALE * FP8_SCALE)

_CACHE = {}


def _build_graph():
    nc = bacc.Bacc("TRN2", target_bir_lowering=False, debug=False, num_devices=N_CORES)
    z = nc.declare_dram_parameter("z", [NB, 128, KT, BAND], mybir.dt.float8e4, isOutput=False)
    rowacc_d = nc.declare_dram_parameter("rowacc", [128, 2, 4, 4], mybir.dt.float32, isOutput=True)
    diagacc_d = nc.declare_dram_parameter("diagacc", [128, 2, 4], mybir.dt.float32, isOutput=True)
    colsum_d = nc.declare_dram_parameter("colsum", [1, 15, BAND], mybir.dt.float32, isOutput=True)

    fp32 = mybir.dt.float32
    bf16 = mybir.dt.bfloat16
    fp8e4 = mybir.dt.float8e4
    fp8e5 = mybir.dt.float8e5
    AF = mybir.ActivationFunctionType
    AX = mybir.AxisListType.X
    DR = mybir.MatmulPerfMode.DoubleRow

    with tile.TileContext(nc) as tc, ExitStack() as ctx:
        zpool = ctx.enter_context(tc.tile_pool(name="z", bufs=1))
        consts = ctx.enter_context(tc.tile_pool(name="consts", bufs=1))
        pspool = ctx.enter_context(tc.tile_pool(name="ps", bufs=3, space="PSUM"))
        cspool = ctx.enter_context(tc.tile_pool(name="cs", bufs=2, space="PSUM"))
        expool = ctx.enter_context(tc.tile_pool(name="ex", bufs=2))
        exdpool = ctx.enter_context(tc.tile_pool(name="exd", bufs=2))
        outpool = ctx.enter_context(tc.tile_pool(name="outp", bufs=1))

        # stage z into SBUF: one [128, KT, 512] fp8 tile per band slot, all on
        # the sync HWDGE queue in slot order so slot 0 (the first block's only
        # dependency) lands first; each tile is split into two dma_starts so
        # the transfer fans out over more DGE queues
        zc = []
        for s in range(NB):
            t = zpool.tile([128, KT, BAND], fp8e4, tag=f"zc{s}", name=f"zc{s}")
            nc.sync.dma_start(out=t[:, 0 : KT // 2, :], in_=z[s, :, 0 : KT // 2, :])
            nc.sync.dma_start(out=t[:, KT // 2 : KT, :], in_=z[s, :, KT // 2 : KT, :])
            zc.append(t)

        # warm-up burst: dummy matmuls keep the PE busy through the HAM
        # activity window while the first z tile is in flight, so the real
        # matmul stream starts un-throttled (2.4 GHz, not 1.2)
        warm = consts.tile([128, 64], fp8e4, tag="warm")
        nc.gpsimd.memset(warm[:], 0.0)
        warmps = cspool.tile([128, BAND], fp32, tag="cs", name="warmps")
        for _ in range(36):
            nc.tensor.matmul(warmps[0:64, 0:64], lhsT=warm[:], rhs=warm[:],
                             start=True, stop=True)

        # constants: 1-identity (self mask, bf16) and a fp8 ones column for
        # the DoubleRow column-sum matmuls
        aeye = consts.tile([128, 128], bf16, tag="aeye")
        nc.gpsimd.memset(aeye[:], 1.0)
        nc.gpsimd.affine_select(
            out=aeye[:],
            in_=aeye[:],
            compare_op=mybir.AluOpType.not_equal,
            fill=0.0,
            base=0,
            pattern=[[-1, 128]],
            channel_multiplier=1,
        )
        ones8 = consts.tile([128, 2, 128], fp8e5, tag="ones8")
        nc.gpsimd.memset(ones8[:], 1.0)

        rowacc = outpool.tile([128, 2, 4, 4], fp32, tag="rowacc")
        diagacc = outpool.tile([128, 2, 4], fp32, tag="diagacc")
        cs_sb = outpool.tile([128, 15, BAND], fp32, tag="cs_sb")

        def mm_chain(ps_slice, L, s, mm):
            # [128 rows, 512 cols] block tile: 4 DoubleRow matmuls, K=1024
            for kp in range(4):
                nc.tensor.matmul(
                    ps_slice,
                    lhsT=zc[L][:, 2 * kp : 2 * kp + 2, 128 * mm : 128 * mm + 128],
                    rhs=zc[s][:, 2 * kp : 2 * kp + 2, :],
                    start=(kp == 0),
                    stop=(kp == 3),
                    perf_mode=DR,
                )

        # deferred column-sum emission: tensor-engine instructions execute in
        # program order, so the ones-matmuls (which wait on the scalar engine's
        # exp) are emitted after the NEXT pair's first chains to avoid PE stalls
        pending_cs = []
        cs_state = {"idx": 0, "cur": None}

        def flush_cs():
            for fn in pending_cs:
                fn()
            pending_cs.clear()

        def emit_cs(exq, h):
            # ones-matmul along the partition axis: every output row equals
            # the column sums of the block's 512 rows; row 0 is kept
            ci = cs_state["idx"]
            cs_state["idx"] += 1
            cur = cspool.tile([128, BAND], fp32, tag="cs", name="cs")
            nc.tensor.matmul(
                cur[:],
                lhsT=ones8[:],
                rhs=exq[:, 0:2, h, :],
                start=True,
                stop=False,
                perf_mode=DR,
            )
            nc.tensor.matmul(
                cur[:],
                lhsT=ones8[:],
                rhs=exq[:, 2:4, h, :],
                start=False,
                stop=True,
                perf_mode=DR,
            )
            nc.vector.tensor_copy(cs_sb[0:1, ci, :], cur[0:1, :])

        def do_diag(d, L):
            # self block (slot L, slot L): exp, mask self-diagonal, row-reduce.
            # exp is issued per 512-col half so the DVE mask/reduce of half h
            # overlaps the exp of half h+1 (shortens the kernel-tail chain)
            for mp in range(2):
                ps = pspool.tile([128, 2, BAND], fp32, tag="ps", name="ps")
                for h in range(2):
                    mm_chain(ps[:, h, :], L, L, 2 * mp + h)
                if mp == 0:
                    flush_cs()
                exd = exdpool.tile([128, 2, BAND], bf16, tag="exd")
                for h in range(2):
                    mm = 2 * mp + h
                    nc.scalar.activation(
                        out=exd[:, h, :], in_=ps[:, h, :], func=AF.Exp, bias=0.0, scale=MM_SCALE
                    )
                    nc.vector.tensor_mul(
                        exd[:, h, 128 * mm : 128 * mm + 128],
                        exd[:, h, 128 * mm : 128 * mm + 128],
                        aeye[:],
                    )
                    nc.vector.reduce_sum(diagacc[:, d, mm : mm + 1], exd[:, h, :], axis=AX)

        def do_pair(d, L, slot, blocks):
            exq = expool.tile([128, 4, 2, BAND], fp8e5, tag="exq")
            if len(blocks) == 2:
                for mm in range(4):
                    ps = pspool.tile([128, 2, BAND], fp32, tag="ps", name="ps")
                    for h, s_ in enumerate(blocks):
                        mm_chain(ps[:, h, :], L, s_, mm)
                    if mm == 1:
                        flush_cs()
                    # fused exp + row-sum over both blocks' 1024 cols
                    nc.scalar.activation(
                        out=exq[:, mm, :, :], in_=ps[:], func=AF.Exp, bias=0.0,
                        scale=MM_SCALE, accum_out=rowacc[:, d, mm, slot : slot + 1],
                    )
            else:
                for mp in range(2):
                    ps = pspool.tile([128, 2, BAND], fp32, tag="ps", name="ps")
                    for h in range(2):
                        mm_chain(ps[:, h, :], L, blocks[0], 2 * mp + h)
                    if mp == 0:
                        flush_cs()
                    for h in range(2):
                        mm = 2 * mp + h
                        nc.scalar.activation(
                            out=exq[:, mm, 0, :], in_=ps[:, h, :], func=AF.Exp, bias=0.0,
                            scale=MM_SCALE, accum_out=rowacc[:, d, mm, slot : slot + 1],
                        )
            for h in range(len(blocks)):
                pending_cs.append(lambda exq=exq, h=h: emit_cs(exq, h))

        do_diag(0, 0)
        do_pair(0, 0, 0, [1, 2])
        do_pair(0, 0, 1, [3, 4])
        do_pair(0, 0, 2, [5, 6])
        do_pair(0, 0, 3, [7, 8])
        do_pair(1, 8, 0, [9, 10])
        do_pair(1, 8, 1, [11, 12])
        do_pair(1, 8, 2, [13, 14])
        do_pair(1, 8, 3, [15])
        do_diag(1, 8)
        flush_cs()

        nc.sync.dma_start(out=rowacc_d[:], in_=rowacc[:])
        nc.sync.dma_start(out=diagacc_d[:], in_=diagacc[:])
        nc.sync.dma_start(out=colsum_d[:], in_=cs_sb[0:1, :, :])

    nc.compile()
    return nc


def _make_in_maps(z1: np.ndarray, z2: np.ndarray):
    z = np.concatenate([z1, z2], axis=0)          # [8192, 1024] f32
    zt = (z.T * FP8_SCALE).astype(np.float32)     # [1024, 8192]
    # [NB, 128, KT, BAND] band-major fp8 tiles: band, k-within-tile, k-tile, col
    zb = np.ascontiguousarray(
        zt.reshape(KT, 128, NB, BAND).transpose(2, 1, 0, 3)
    ).astype(ml_dtypes.float8_e4m3)
    return [
        {"z": np.ascontiguousarray(zb[[(c + s) % NB for s in range(NB)]])}
        for c in range(N_CORES)
    ]


def kernel(z1: np.ndarray, z2: np.ndarray) -> np.ndarray:
    assert z1.shape == (N, D) and z2.shape == (N, D)
    in_maps = _make_in_maps(z1, z2)

    if "nc" not in _CACHE:
        _CACHE["nc"] = _build_graph()
    res = run_bass_kernel_spmd(_CACHE["nc"], in_maps, core_ids=list(range(N_CORES)))

    S = np.zeros(ROWS, np.float64)
    for c in range(N_CORES):
        r = res.results[c]
        ra = np.asarray(r["rowacc"], dtype=np.float64)    # [128, 2, 4, 4]
        da = np.asarray(r["diagacc"], dtype=np.float64)   # [128, 2, 4]
        cs = np.asarray(r["colsum"], dtype=np.float64)[0]  # [15, 512]
        for d, L in ((0, 0), (1, 8)):
            b = (c + L) % NB
            vals = ra[:, d, :, :].sum(axis=2) + da[:, d, :]   # [128 p, 4 m]
            S[BAND * b : BAND * (b + 1)] += vals.T.reshape(BAND)
        for ci, s in enumerate(list(range(1, 9)) + list(range(9, 16))):
            b = (c + s) % NB
            S[BAND * b : BAND * (b + 1)] += cs[ci, :]

    pos = (z1.astype(np.float64) * z2.astype(np.float64)).sum(axis=1) / TEMP
    loss = np.log(S).mean() - pos.mean()
    return np.asarray(loss, dtype=np.float32)
